# revision 25
# baseline (speedup 1.0000x reference)
# Trainium2 Bass kernel for nn_LocalCrossAttentionModule.
#
# Math: softmax over a size-1 axis is identically 1, so q/k (and x_query,
# Wq, bq, Wk, bk) never affect the output. The module reduces to, per
# 5x5 patch p (576 of them = 4 batch x 12x12 grid, stride 36):
#   kvf_p  = flatten(x_kv patch)                  (3200,)
#   v_p    = Wv @ kvf_p + bv                      (1600,) viewed as (64, 5, 5)
#   z_p    = conv_w @ v_p[:, s] + conv_b          (128,) per pixel s in 5x5
# z_p is scattered into an otherwise-constant (conv_b) output map.
#
# Sharding: the 25 patch pixels s are split across 8 cores (4 slots each,
# 7 junk/dup slots). Every core sees all 576 patches as the matmul moving
# dim (2 chunks of 288 >= 256 keeps float32r matmuls at full rate).
# Host does layout only: patch gather, weight permutation/transpose,
# final scatter into the conv_b-filled canvas.

import numpy as np

B = 4
CKV = 128
HW_ = 432
E = 2
PP = 5          # patch side
STRIDE = 36
PI = 12         # patch grid side
NP = B * PI * PI   # 576 patches
KF = CKV * PP * PP  # 3200 kv features per patch
KT = KF // 128      # 25 contraction tiles
OUT = 64
O2 = 128
SLOTS = 4
M = SLOTS * OUT    # 256 v-features per core
NCHUNK = 288       # patch chunk (2 x 288 = 576)
NCORES = 8

DTYPE = "f32r"     # "f32r" (accurate) or "bf16" (half the DMA bytes)

# pixel-slot assignment: cores 0-6 own 3 pixels (4th slot duplicates the
# first), core 7 owns 4.
S_LISTS = [[3 * c, 3 * c + 1, 3 * c + 2, 3 * c] for c in range(7)]
S_LISTS.append([21, 22, 23, 24])
VALID = [3] * 7 + [4]

_PROGRAM = {}


def _build_program(dtype=DTYPE):
    import concourse.mybir as mybir
    from concourse import bacc
    from concourse.tile import TileContext

    f32 = mybir.dt.float32
    mm_dt = mybir.dt.float32r if dtype == "f32r" else mybir.dt.bfloat16
    # matmul-2 operand dtype: DVE cannot produce float32r, so f32r mode
    # runs the (tiny) second matmul in plain fp32
    v_dt = f32 if dtype == "f32r" else mybir.dt.bfloat16

    WKC = M + NP  # 832 cols per k-tile: [w(256) | kvf(576)]

    nc = bacc.Bacc()
    wk_d = nc.declare_dram_parameter("wk", [128, KT, WKC], mm_dt, isOutput=False)
    cwbc_d = nc.declare_dram_parameter("cwbc", [128, 131], f32, isOutput=False)
    z_d = nc.declare_dram_parameter("zout", [128, SLOTS, NP], f32, isOutput=True)

    with TileContext(nc) as tc:
        with (
            tc.tile_pool(name="consts", bufs=1) as cpool,
            tc.tile_pool(name="wbig", bufs=1) as wpool,
            tc.tile_pool(name="vbuf", bufs=1) as vpool,
            tc.tile_pool(name="zbuf", bufs=1) as zpool,
            tc.tile_pool(name="ps1", bufs=1, space="PSUM") as ps1,
            tc.tile_pool(name="ps2", bufs=3, space="PSUM") as ps2,
            tc.tile_pool(name="ps0", bufs=1, space="PSUM") as ps0,
        ):
            # PE warm-up: dummy matmuls on a zeroed scratch tile keep the
            # PE_HAM activity window busy from t~0 so real matmuls run at
            # 2.4 GHz instead of the cold 1.2 GHz
            warm_t = cpool.tile([128, 512], f32, name="warm_t")
            nc.gpsimd.memset(warm_t[:], 0.0)
            wps = ps0.tile([128, 512], f32, name="wps")
            for _ in range(6):
                nc.tensor.matmul(
                    wps[:], lhsT=warm_t[:, 0:128], rhs=warm_t[:],
                    start=True, stop=True,
                )

            cwbc_t = cpool.tile([128, 131], f32, name="cwbc_t")
            nc.sync.dma_start(cwbc_t[:], cwbc_d[:])
            # DVE-produced copy of conv_w.T so matmul-2 waits only on DVE
            cw_t = cpool.tile([128, 128], v_dt, name="cw_t")
            nc.vector.tensor_copy(cw_t[:], cwbc_t[:, 0:128])

            wk_t = wpool.tile([128, KT, WKC], mm_dt, name="wk_t")
            # chunked loads, small first so the first matmul starts early
            sizes = [1, 1, 2, 2, 3, 3, 3, 3, 3, 4]
            lo = 0
            for sz in sizes:
                nc.sync.dma_start(wk_t[:, lo:lo + sz, :], wk_d[:, lo:lo + sz, :])
                lo += sz

            # matmul 1: V[f, n] = sum_j WvT[j, f] * KVF_T[j, n]
            ps_v = [
                [ps1.tile([128, NCHUNK], f32, name=f"psv{m}{n}") for n in range(2)]
                for m in range(2)
            ]
            for k in range(KT):
                for m in range(2):
                    for n in range(2):
                        nc.tensor.matmul(
                            ps_v[m][n][:],
                            lhsT=wk_t[:, k, m * 128:(m + 1) * 128],
                            rhs=wk_t[:, k, M + n * NCHUNK:M + (n + 1) * NCHUNK],
                            start=(k == 0),
                            stop=(k == KT - 1),
                        )
                # keep-warm filler: PE would otherwise idle waiting for the
                # next k-tile DMA, letting PE_HAM throttle the clock to 1.2GHz
                if k % 2 == 0:
                    nc.tensor.matmul(
                        wps[:], lhsT=warm_t[:, 0:128], rhs=warm_t[:],
                        start=True, stop=True,
                    )

            # V to SBUF (+bv), zero-padded to 128 partitions per pixel-slot
            v_t = []
            for s in range(SLOTS):
                vt = vpool.tile([128, NP], v_dt, name=f"vt{s}")
                nc.vector.memset(vt[64:128, :], 0.0)
                v_t.append(vt)
            for m in range(2):
                for n in range(2):
                    for h in range(2):
                        s = 2 * m + h
                        nc.vector.tensor_tensor(
                            out=v_t[s][0:64, n * NCHUNK:(n + 1) * NCHUNK],
                            in0=ps_v[m][n][h * 64:(h + 1) * 64, :],
                            in1=cwbc_t[h * 64:(h + 1) * 64, 128 + m:129 + m]
                            .to_broadcast((64, NCHUNK)),
                            op=mybir.AluOpType.add,
                        )

            # keep PE busy while DVE copies V out of PSUM, so matmul-2 does
            # not start on a throttled clock
            for _ in range(3):
                nc.tensor.matmul(
                    wps[:], lhsT=warm_t[:, 0:128], rhs=warm_t[:],
                    start=True, stop=True,
                )

            # matmul 2: z[o2, n] = sum_o conv_w[o2, o] * V[s*64+o, n]
            z_t = zpool.tile([128, SLOTS, NP], f32, name="z_t")
            for s in range(SLOTS):
                for n in range(2):
                    nsl = slice(n * NCHUNK, (n + 1) * NCHUNK)
                    psz = ps2.tile([128, NCHUNK], f32, name="psz")
                    nc.tensor.matmul(
                        psz[:],
                        lhsT=cw_t[:],
                        rhs=v_t[s][:, nsl],
                        start=True,
                        stop=True,
                    )
                    nc.vector.tensor_tensor(
                        out=z_t[:, s, nsl],
                        in0=psz[:],
                        in1=cwbc_t[:, 130:131].to_broadcast((128, NCHUNK)),
                        op=mybir.AluOpType.add,
                    )
                # store each pixel-slot as soon as it is ready
                nc.sync.dma_start(z_d[:, s, :], z_t[:, s, :])
    nc.finalize()
    return nc


def _get_program(dtype=DTYPE):
    if dtype not in _PROGRAM:
        _PROGRAM[dtype] = _build_program(dtype)
    return _PROGRAM[dtype]


def _round_fp32r(a):
    """Round fp32 array to the FP32R grid (12-bit mantissa): (u+0x800)&~0xfff."""
    u = np.ascontiguousarray(a, dtype=np.float32).view(np.uint32)
    u = (u + np.uint32(0x800)) & np.uint32(0xFFFFF000)
    return u.view(np.float32)


def _mm_cast(a, dtype):
    if dtype == "f32r":
        return _round_fp32r(a)
    import ml_dtypes

    return np.ascontiguousarray(a, dtype=np.float32).astype(ml_dtypes.bfloat16)


def _prep_in_maps(x_kv, Wv, bv, conv_w, conv_b, dtype=DTYPE):
    """Host-side shard/layout prep. Returns list of per-core input dicts."""
    x_kv = np.ascontiguousarray(np.asarray(x_kv, dtype=np.float32))
    Wv = np.asarray(Wv, dtype=np.float32)
    bv = np.asarray(bv, dtype=np.float32)
    conv_w = np.asarray(conv_w, dtype=np.float32)
    conv_b = np.asarray(conv_b, dtype=np.float32)

    # gather all 5x5 patches (padded coords: top-left of patch (pi,pj) is
    # original coords (pi*36-2, pj*36-2))
    pad = np.zeros((B, CKV, HW_ + 2 * E, HW_ + 2 * E), np.float32)
    pad[:, :, E:HW_ + E, E:HW_ + E] = x_kv
    r = (np.arange(PI)[:, None] * STRIDE + np.arange(PP)).ravel()  # (60,)
    g = pad[:, :, r[:, None], r[None, :]]                # (B, C, 60, 60)
    g = g.reshape(B, CKV, PI, PP, PI, PP)
    # feature j = c*25 + pr*5 + pc ; patch n = b*144 + pi*12 + pj
    kvf_t = g.transpose(1, 3, 5, 0, 2, 4).reshape(KF, NP)   # (3200, 576)
    # device layout [partition, k-tile, patch]
    kvf_arr = kvf_t.reshape(KT, 128, NP).transpose(1, 0, 2)

    cw = np.zeros((128, 128), np.float32)
    cw[:OUT, :] = conv_w.T  # cw[o, o2] = conv_w[o2, o]

    in_maps = []
    for c in range(NCORES):
        perm = np.array(
            [o * PP * PP + s for s in S_LISTS[c] for o in range(OUT)], np.int64
        )
        wv_t = Wv[perm].T                      # (3200, 256)
        wv_arr = wv_t.reshape(KT, 128, M).transpose(1, 0, 2)
        # single blob: per k-tile [w(256) | kvf(576)]
        wk = np.concatenate([wv_arr, kvf_arr], axis=2)  # (128, 25, 832)
        wk = _mm_cast(wk, dtype)
        # f32 consts blob: [cw(128) | bv(2) | cb(1)]
        cwbc = np.empty((128, 131), np.float32)
        cwbc[:, 0:128] = cw
        cwbc[:, 128:130] = bv[perm].reshape(2, 128).T
        cwbc[:, 130] = conv_b
        in_maps.append({"wk": wk, "cwbc": cwbc})
    return in_maps


def _assemble(z_list, conv_b, out_dtype=np.float32):
    """Scatter per-core z outputs into the full (B, 128, 432, 432) map."""
    conv_b = np.asarray(conv_b, dtype=np.float32)
    y = np.empty((B, O2, HW_, HW_), np.float32)
    y[:] = conv_b.reshape(1, O2, 1, 1)
    base = np.arange(PI) * STRIDE
    for c in range(NCORES):
        z = z_list[c]  # (128, SLOTS, 576)
        for t in range(VALID[c]):
            s = S_LISTS[c][t]
            pr, pc = divmod(s, PP)
            blk = z[:, t, :].reshape(O2, B, PI, PI).transpose(1, 0, 2, 3)
            y[:, :, (base + pr)[:, None], (base + pc)[None, :]] = blk
    return y.astype(out_dtype, copy=False)


def _run(inputs, trace=False, trace_kwargs=None, dtype=DTYPE):
    from concourse.bass_utils import run_bass_kernel_spmd

    in_maps = _prep_in_maps(
        inputs["x_kv"], inputs["Wv"], inputs["bv"],
        inputs["conv_w"], inputs["conv_b"], dtype=dtype,
    )
    nc = _get_program(dtype)
    kw = {}
    if trace:
        kw["trace"] = True
        if trace_kwargs:
            kw.update(trace_kwargs)
    res = run_bass_kernel_spmd(nc, in_maps, list(range(NCORES)), **kw)
    z_list = [res.results[c]["zout"] for c in range(NCORES)]
    out = _assemble(z_list, inputs["conv_b"])
    return out, res


def kernel(**inputs):
    out, _ = _run(inputs, trace=False)
    return out


# revision 27
# speedup vs baseline: 1.0680x; 1.0680x over previous
# Trainium2 Bass kernel for nn_LocalCrossAttentionModule.
#
# Math: softmax over a size-1 axis is identically 1, so q/k (and x_query,
# Wq, bq, Wk, bk) never affect the output. The module reduces to, per
# 5x5 patch p (576 of them = 4 batch x 12x12 grid, stride 36):
#   kvf_p  = flatten(x_kv patch)                  (3200,)
#   v_p    = Wv @ kvf_p + bv                      (1600,) viewed as (64, 5, 5)
#   z_p    = conv_w @ v_p[:, s] + conv_b          (128,) per pixel s in 5x5
# z_p is scattered into an otherwise-constant (conv_b) output map.
#
# Sharding: the 25 patch pixels s are split across 8 cores (4 slots each,
# 7 junk/dup slots). Every core sees all 576 patches as the matmul moving
# dim (2 chunks of 288 >= 256 keeps float32r matmuls at full rate).
# Host does layout only: patch gather, weight permutation/transpose,
# final scatter into the conv_b-filled canvas.

import numpy as np

B = 4
CKV = 128
HW_ = 432
E = 2
PP = 5          # patch side
STRIDE = 36
PI = 12         # patch grid side
NP = B * PI * PI   # 576 patches
KF = CKV * PP * PP  # 3200 kv features per patch
KT = KF // 128      # 25 contraction tiles
OUT = 64
O2 = 128
SLOTS = 4
M = SLOTS * OUT    # 256 v-features per core
NCHUNK = 288       # patch chunk (2 x 288 = 576)
NCORES = 8

DTYPE = "f32r"     # "f32r" (accurate) or "bf16" (half the DMA bytes)

# pixel-slot assignment: cores 0-6 own 3 pixels (4th slot duplicates the
# first), core 7 owns 4.
S_LISTS = [[3 * c, 3 * c + 1, 3 * c + 2, 3 * c] for c in range(7)]
S_LISTS.append([21, 22, 23, 24])
VALID = [3] * 7 + [4]

_PROGRAM = {}


def _build_program(dtype=DTYPE):
    import concourse.mybir as mybir
    from concourse import bacc
    from concourse.tile import TileContext

    f32 = mybir.dt.float32
    mm_dt = mybir.dt.float32r if dtype == "f32r" else mybir.dt.bfloat16
    # matmul-2 operand dtype: DVE cannot produce float32r, so f32r mode
    # runs the (tiny) second matmul in plain fp32
    v_dt = f32 if dtype == "f32r" else mybir.dt.bfloat16

    WKC = M + NP  # 832 cols per k-tile: [w(256) | kvf(576)]

    nc = bacc.Bacc()
    wk_d = nc.declare_dram_parameter("wk", [128, KT, WKC], mm_dt, isOutput=False)
    cwbc_d = nc.declare_dram_parameter("cwbc", [128, 131], f32, isOutput=False)
    z_d = nc.declare_dram_parameter("zout", [128, SLOTS, NP], f32, isOutput=True)

    with TileContext(nc) as tc:
        with (
            tc.tile_pool(name="consts", bufs=1) as cpool,
            tc.tile_pool(name="wbig", bufs=1) as wpool,
            tc.tile_pool(name="vbuf", bufs=1) as vpool,
            tc.tile_pool(name="zbuf", bufs=1) as zpool,
            tc.tile_pool(name="ps1", bufs=1, space="PSUM") as ps1,
            tc.tile_pool(name="ps2", bufs=3, space="PSUM") as ps2,
            tc.tile_pool(name="ps0", bufs=1, space="PSUM") as ps0,
        ):
            # PE warm-up: dummy matmuls on a zeroed scratch tile keep the
            # PE_HAM activity window busy from t~0 so real matmuls run at
            # 2.4 GHz instead of the cold 1.2 GHz
            warm_t = cpool.tile([128, 512], f32, name="warm_t")
            nc.gpsimd.memset(warm_t[:], 0.0)
            wps = ps0.tile([128, 512], f32, name="wps")
            for _ in range(6):
                nc.tensor.matmul(
                    wps[:], lhsT=warm_t[:, 0:128], rhs=warm_t[:],
                    start=True, stop=True,
                )

            cwbc_t = cpool.tile([128, 131], f32, name="cwbc_t")
            nc.sync.dma_start(cwbc_t[:], cwbc_d[:])
            # DVE-produced copy of conv_w.T so matmul-2 waits only on DVE
            cw_t = cpool.tile([128, 128], v_dt, name="cw_t")
            nc.vector.tensor_copy(cw_t[:], cwbc_t[:, 0:128])

            wk_t = wpool.tile([128, KT, WKC], mm_dt, name="wk_t")
            # chunked loads, small first so the first matmul starts early
            sizes = [1, 1, 2, 2, 3, 3, 3, 3, 3, 4]
            lo = 0
            for sz in sizes:
                nc.sync.dma_start(wk_t[:, lo:lo + sz, :], wk_d[:, lo:lo + sz, :])
                lo += sz

            # matmul 1: V[f, n] = sum_j WvT[j, f] * KVF_T[j, n]
            ps_v = [
                [ps1.tile([128, NCHUNK], f32, name=f"psv{m}{n}") for n in range(2)]
                for m in range(2)
            ]
            for k in range(KT):
                for m in range(2):
                    for n in range(2):
                        nc.tensor.matmul(
                            ps_v[m][n][:],
                            lhsT=wk_t[:, k, m * 128:(m + 1) * 128],
                            rhs=wk_t[:, k, M + n * NCHUNK:M + (n + 1) * NCHUNK],
                            start=(k == 0),
                            stop=(k == KT - 1),
                        )
                # keep-warm filler: PE would otherwise idle waiting for the
                # next k-tile DMA, letting PE_HAM throttle the clock to 1.2GHz
                if k % 2 == 0:
                    nc.tensor.matmul(
                        wps[:], lhsT=warm_t[:, 0:128], rhs=warm_t[:],
                        start=True, stop=True,
                    )

            # V to SBUF (+bv), zero-padded to 128 partitions per pixel-slot
            v_t = []
            for s in range(SLOTS):
                vt = vpool.tile([128, NP], v_dt, name=f"vt{s}")
                nc.vector.memset(vt[64:128, :], 0.0)
                v_t.append(vt)
            for m in range(2):
                for n in range(2):
                    for h in range(2):
                        s = 2 * m + h
                        nc.vector.tensor_tensor(
                            out=v_t[s][0:64, n * NCHUNK:(n + 1) * NCHUNK],
                            in0=ps_v[m][n][h * 64:(h + 1) * 64, :],
                            in1=cwbc_t[h * 64:(h + 1) * 64, 128 + m:129 + m]
                            .to_broadcast((64, NCHUNK)),
                            op=mybir.AluOpType.add,
                        )

            # matmul 2: z[o2, n] = sum_o conv_w[o2, o] * V[s*64+o, n]
            z_t = zpool.tile([128, SLOTS, NP], f32, name="z_t")
            for s in range(SLOTS):
                for n in range(2):
                    nsl = slice(n * NCHUNK, (n + 1) * NCHUNK)
                    psz = ps2.tile([128, NCHUNK], f32, name="psz")
                    nc.tensor.matmul(
                        psz[:],
                        lhsT=cw_t[:],
                        rhs=v_t[s][:, nsl],
                        start=True,
                        stop=True,
                    )
                    nc.vector.tensor_tensor(
                        out=z_t[:, s, nsl],
                        in0=psz[:],
                        in1=cwbc_t[:, 130:131].to_broadcast((128, NCHUNK)),
                        op=mybir.AluOpType.add,
                    )
                    # store each chunk as soon as it is ready
                    nc.sync.dma_start(z_d[:, s, nsl], z_t[:, s, nsl])
    nc.finalize()
    return nc


def _get_program(dtype=DTYPE):
    if dtype not in _PROGRAM:
        _PROGRAM[dtype] = _build_program(dtype)
    return _PROGRAM[dtype]


def _round_fp32r(a):
    """Round fp32 array to the FP32R grid (12-bit mantissa): (u+0x800)&~0xfff."""
    u = np.ascontiguousarray(a, dtype=np.float32).view(np.uint32)
    u = (u + np.uint32(0x800)) & np.uint32(0xFFFFF000)
    return u.view(np.float32)


def _mm_cast(a, dtype):
    if dtype == "f32r":
        return _round_fp32r(a)
    import ml_dtypes

    return np.ascontiguousarray(a, dtype=np.float32).astype(ml_dtypes.bfloat16)


def _prep_in_maps(x_kv, Wv, bv, conv_w, conv_b, dtype=DTYPE):
    """Host-side shard/layout prep. Returns list of per-core input dicts."""
    x_kv = np.ascontiguousarray(np.asarray(x_kv, dtype=np.float32))
    Wv = np.asarray(Wv, dtype=np.float32)
    bv = np.asarray(bv, dtype=np.float32)
    conv_w = np.asarray(conv_w, dtype=np.float32)
    conv_b = np.asarray(conv_b, dtype=np.float32)

    # gather all 5x5 patches (padded coords: top-left of patch (pi,pj) is
    # original coords (pi*36-2, pj*36-2))
    pad = np.zeros((B, CKV, HW_ + 2 * E, HW_ + 2 * E), np.float32)
    pad[:, :, E:HW_ + E, E:HW_ + E] = x_kv
    r = (np.arange(PI)[:, None] * STRIDE + np.arange(PP)).ravel()  # (60,)
    g = pad[:, :, r[:, None], r[None, :]]                # (B, C, 60, 60)
    g = g.reshape(B, CKV, PI, PP, PI, PP)
    # feature j = c*25 + pr*5 + pc ; patch n = b*144 + pi*12 + pj
    kvf_t = g.transpose(1, 3, 5, 0, 2, 4).reshape(KF, NP)   # (3200, 576)
    # device layout [partition, k-tile, patch]
    kvf_arr = kvf_t.reshape(KT, 128, NP).transpose(1, 0, 2)

    cw = np.zeros((128, 128), np.float32)
    cw[:OUT, :] = conv_w.T  # cw[o, o2] = conv_w[o2, o]

    in_maps = []
    for c in range(NCORES):
        perm = np.array(
            [o * PP * PP + s for s in S_LISTS[c] for o in range(OUT)], np.int64
        )
        wv_t = Wv[perm].T                      # (3200, 256)
        wv_arr = wv_t.reshape(KT, 128, M).transpose(1, 0, 2)
        # single blob: per k-tile [w(256) | kvf(576)]
        wk = np.concatenate([wv_arr, kvf_arr], axis=2)  # (128, 25, 832)
        wk = _mm_cast(wk, dtype)
        # f32 consts blob: [cw(128) | bv(2) | cb(1)]
        cwbc = np.empty((128, 131), np.float32)
        cwbc[:, 0:128] = cw
        cwbc[:, 128:130] = bv[perm].reshape(2, 128).T
        cwbc[:, 130] = conv_b
        in_maps.append({"wk": wk, "cwbc": cwbc})
    return in_maps


def _assemble(z_list, conv_b, out_dtype=np.float32):
    """Scatter per-core z outputs into the full (B, 128, 432, 432) map."""
    conv_b = np.asarray(conv_b, dtype=np.float32)
    y = np.empty((B, O2, HW_, HW_), np.float32)
    y[:] = conv_b.reshape(1, O2, 1, 1)
    base = np.arange(PI) * STRIDE
    for c in range(NCORES):
        z = z_list[c]  # (128, SLOTS, 576)
        for t in range(VALID[c]):
            s = S_LISTS[c][t]
            pr, pc = divmod(s, PP)
            blk = z[:, t, :].reshape(O2, B, PI, PI).transpose(1, 0, 2, 3)
            y[:, :, (base + pr)[:, None], (base + pc)[None, :]] = blk
    return y.astype(out_dtype, copy=False)


def _run(inputs, trace=False, trace_kwargs=None, dtype=DTYPE):
    from concourse.bass_utils import run_bass_kernel_spmd

    in_maps = _prep_in_maps(
        inputs["x_kv"], inputs["Wv"], inputs["bv"],
        inputs["conv_w"], inputs["conv_b"], dtype=dtype,
    )
    nc = _get_program(dtype)
    kw = {}
    if trace:
        kw["trace"] = True
        if trace_kwargs:
            kw.update(trace_kwargs)
    res = run_bass_kernel_spmd(nc, in_maps, list(range(NCORES)), **kw)
    z_list = [res.results[c]["zout"] for c in range(NCORES)]
    out = _assemble(z_list, inputs["conv_b"])
    return out, res


def kernel(**inputs):
    out, _ = _run(inputs, trace=False)
    return out


# revision 28
# speedup vs baseline: 1.1155x; 1.0444x over previous
# Trainium2 Bass kernel for nn_LocalCrossAttentionModule.
#
# Math: softmax over a size-1 axis is identically 1, so q/k (and x_query,
# Wq, bq, Wk, bk) never affect the output. The module reduces to, per
# 5x5 patch p (576 of them = 4 batch x 12x12 grid, stride 36):
#   kvf_p  = flatten(x_kv patch)                  (3200,)
#   v_p    = Wv @ kvf_p + bv                      (1600,) viewed as (64, 5, 5)
#   z_p    = conv_w @ v_p[:, s] + conv_b          (128,) per pixel s in 5x5
# z_p is scattered into an otherwise-constant (conv_b) output map.
#
# Sharding: the 25 patch pixels s are split across 8 cores (4 slots each,
# 7 junk/dup slots). Every core sees all 576 patches as the matmul moving
# dim (2 chunks of 288 >= 256 keeps float32r matmuls at full rate).
# Host does layout only: patch gather, weight permutation/transpose,
# final scatter into the conv_b-filled canvas.

import numpy as np

B = 4
CKV = 128
HW_ = 432
E = 2
PP = 5          # patch side
STRIDE = 36
PI = 12         # patch grid side
NP = B * PI * PI   # 576 patches
KF = CKV * PP * PP  # 3200 kv features per patch
KT = KF // 128      # 25 contraction tiles
OUT = 64
O2 = 128
SLOTS = 4
M = SLOTS * OUT    # 256 v-features per core
NCHUNK = 288       # patch chunk (2 x 288 = 576)
NCORES = 8

DTYPE = "f16"      # "f32r" (most accurate) | "f16" (half DMA bytes, ~5e-4) | "bf16"

# pixel-slot assignment: cores 0-6 own 3 pixels (4th slot duplicates the
# first), core 7 owns 4.
S_LISTS = [[3 * c, 3 * c + 1, 3 * c + 2, 3 * c] for c in range(7)]
S_LISTS.append([21, 22, 23, 24])
VALID = [3] * 7 + [4]

_PROGRAM = {}


def _build_program(dtype=DTYPE):
    import concourse.mybir as mybir
    from concourse import bacc
    from concourse.tile import TileContext

    f32 = mybir.dt.float32
    half = {"bf16": mybir.dt.bfloat16, "f16": mybir.dt.float16}
    mm_dt = mybir.dt.float32r if dtype == "f32r" else half[dtype]
    # matmul-2 operand dtype: DVE cannot produce float32r, so f32r mode
    # runs the (tiny) second matmul in plain fp32
    v_dt = f32 if dtype == "f32r" else half[dtype]

    WKC = M + NP  # 832 cols per k-tile: [w(256) | kvf(576)]

    nc = bacc.Bacc()
    wk_d = nc.declare_dram_parameter("wk", [128, KT, WKC], mm_dt, isOutput=False)
    cwbc_d = nc.declare_dram_parameter("cwbc", [128, 131], f32, isOutput=False)
    z_d = nc.declare_dram_parameter("zout", [128, SLOTS, NP], f32, isOutput=True)

    with TileContext(nc) as tc:
        with (
            tc.tile_pool(name="consts", bufs=1) as cpool,
            tc.tile_pool(name="wbig", bufs=1) as wpool,
            tc.tile_pool(name="vbuf", bufs=1) as vpool,
            tc.tile_pool(name="zbuf", bufs=1) as zpool,
            tc.tile_pool(name="ps1", bufs=1, space="PSUM") as ps1,
            tc.tile_pool(name="ps2", bufs=3, space="PSUM") as ps2,
            tc.tile_pool(name="ps0", bufs=1, space="PSUM") as ps0,
        ):
            # PE warm-up: dummy matmuls on a zeroed scratch tile keep the
            # PE_HAM activity window busy from t~0 so real matmuls run at
            # 2.4 GHz instead of the cold 1.2 GHz
            warm_t = cpool.tile([128, 512], f32, name="warm_t")
            nc.gpsimd.memset(warm_t[:], 0.0)
            wps = ps0.tile([128, 512], f32, name="wps")
            for _ in range(6):
                nc.tensor.matmul(
                    wps[:], lhsT=warm_t[:, 0:128], rhs=warm_t[:],
                    start=True, stop=True,
                )

            cwbc_t = cpool.tile([128, 131], f32, name="cwbc_t")
            nc.sync.dma_start(cwbc_t[:], cwbc_d[:])
            # DVE-produced copy of conv_w.T so matmul-2 waits only on DVE
            cw_t = cpool.tile([128, 128], v_dt, name="cw_t")
            nc.vector.tensor_copy(cw_t[:], cwbc_t[:, 0:128])

            wk_t = wpool.tile([128, KT, WKC], mm_dt, name="wk_t")
            # chunked loads, small first so the first matmul starts early
            sizes = [1, 1, 2, 2, 3, 3, 3, 3, 3, 4]
            lo = 0
            for sz in sizes:
                nc.sync.dma_start(wk_t[:, lo:lo + sz, :], wk_d[:, lo:lo + sz, :])
                lo += sz

            # matmul 1: V[f, n] = sum_j WvT[j, f] * KVF_T[j, n]
            ps_v = [
                [ps1.tile([128, NCHUNK], f32, name=f"psv{m}{n}") for n in range(2)]
                for m in range(2)
            ]
            for k in range(KT):
                for m in range(2):
                    for n in range(2):
                        nc.tensor.matmul(
                            ps_v[m][n][:],
                            lhsT=wk_t[:, k, m * 128:(m + 1) * 128],
                            rhs=wk_t[:, k, M + n * NCHUNK:M + (n + 1) * NCHUNK],
                            start=(k == 0),
                            stop=(k == KT - 1),
                        )
                # keep-warm filler: PE would otherwise idle waiting for the
                # next k-tile DMA, letting PE_HAM throttle the clock to 1.2GHz
                if k % 2 == 0:
                    nc.tensor.matmul(
                        wps[:], lhsT=warm_t[:, 0:128], rhs=warm_t[:],
                        start=True, stop=True,
                    )

            # V to SBUF (+bv), zero-padded to 128 partitions per pixel-slot
            v_t = []
            for s in range(SLOTS):
                vt = vpool.tile([128, NP], v_dt, name=f"vt{s}")
                nc.vector.memset(vt[64:128, :], 0.0)
                v_t.append(vt)
            for m in range(2):
                for n in range(2):
                    for h in range(2):
                        s = 2 * m + h
                        nc.vector.tensor_tensor(
                            out=v_t[s][0:64, n * NCHUNK:(n + 1) * NCHUNK],
                            in0=ps_v[m][n][h * 64:(h + 1) * 64, :],
                            in1=cwbc_t[h * 64:(h + 1) * 64, 128 + m:129 + m]
                            .to_broadcast((64, NCHUNK)),
                            op=mybir.AluOpType.add,
                        )

            # matmul 2: z[o2, n] = sum_o conv_w[o2, o] * V[s*64+o, n]
            z_t = zpool.tile([128, SLOTS, NP], f32, name="z_t")
            for s in range(SLOTS):
                for n in range(2):
                    nsl = slice(n * NCHUNK, (n + 1) * NCHUNK)
                    psz = ps2.tile([128, NCHUNK], f32, name="psz")
                    nc.tensor.matmul(
                        psz[:],
                        lhsT=cw_t[:],
                        rhs=v_t[s][:, nsl],
                        start=True,
                        stop=True,
                    )
                    nc.vector.tensor_tensor(
                        out=z_t[:, s, nsl],
                        in0=psz[:],
                        in1=cwbc_t[:, 130:131].to_broadcast((128, NCHUNK)),
                        op=mybir.AluOpType.add,
                    )
                    # store each chunk as soon as it is ready
                    nc.sync.dma_start(z_d[:, s, nsl], z_t[:, s, nsl])
    nc.finalize()
    return nc


def _get_program(dtype=DTYPE):
    if dtype not in _PROGRAM:
        _PROGRAM[dtype] = _build_program(dtype)
    return _PROGRAM[dtype]


def _round_fp32r(a):
    """Round fp32 array to the FP32R grid (12-bit mantissa): (u+0x800)&~0xfff."""
    u = np.ascontiguousarray(a, dtype=np.float32).view(np.uint32)
    u = (u + np.uint32(0x800)) & np.uint32(0xFFFFF000)
    return u.view(np.float32)


def _mm_cast(a, dtype):
    if dtype == "f32r":
        return _round_fp32r(a)
    if dtype == "f16":
        return np.ascontiguousarray(a, dtype=np.float32).astype(np.float16)
    import ml_dtypes

    return np.ascontiguousarray(a, dtype=np.float32).astype(ml_dtypes.bfloat16)


def _prep_in_maps(x_kv, Wv, bv, conv_w, conv_b, dtype=DTYPE):
    """Host-side shard/layout prep. Returns list of per-core input dicts."""
    x_kv = np.ascontiguousarray(np.asarray(x_kv, dtype=np.float32))
    Wv = np.asarray(Wv, dtype=np.float32)
    bv = np.asarray(bv, dtype=np.float32)
    conv_w = np.asarray(conv_w, dtype=np.float32)
    conv_b = np.asarray(conv_b, dtype=np.float32)

    # gather all 5x5 patches (padded coords: top-left of patch (pi,pj) is
    # original coords (pi*36-2, pj*36-2))
    pad = np.zeros((B, CKV, HW_ + 2 * E, HW_ + 2 * E), np.float32)
    pad[:, :, E:HW_ + E, E:HW_ + E] = x_kv
    r = (np.arange(PI)[:, None] * STRIDE + np.arange(PP)).ravel()  # (60,)
    g = pad[:, :, r[:, None], r[None, :]]                # (B, C, 60, 60)
    g = g.reshape(B, CKV, PI, PP, PI, PP)
    # feature j = c*25 + pr*5 + pc ; patch n = b*144 + pi*12 + pj
    kvf_t = g.transpose(1, 3, 5, 0, 2, 4).reshape(KF, NP)   # (3200, 576)
    # device layout [partition, k-tile, patch]
    kvf_arr = kvf_t.reshape(KT, 128, NP).transpose(1, 0, 2)

    cw = np.zeros((128, 128), np.float32)
    cw[:OUT, :] = conv_w.T  # cw[o, o2] = conv_w[o2, o]

    in_maps = []
    for c in range(NCORES):
        perm = np.array(
            [o * PP * PP + s for s in S_LISTS[c] for o in range(OUT)], np.int64
        )
        wv_t = Wv[perm].T                      # (3200, 256)
        wv_arr = wv_t.reshape(KT, 128, M).transpose(1, 0, 2)
        # single blob: per k-tile [w(256) | kvf(576)]
        wk = np.concatenate([wv_arr, kvf_arr], axis=2)  # (128, 25, 832)
        wk = _mm_cast(wk, dtype)
        # f32 consts blob: [cw(128) | bv(2) | cb(1)]
        cwbc = np.empty((128, 131), np.float32)
        cwbc[:, 0:128] = cw
        cwbc[:, 128:130] = bv[perm].reshape(2, 128).T
        cwbc[:, 130] = conv_b
        in_maps.append({"wk": wk, "cwbc": cwbc})
    return in_maps


def _assemble(z_list, conv_b, out_dtype=np.float32):
    """Scatter per-core z outputs into the full (B, 128, 432, 432) map."""
    conv_b = np.asarray(conv_b, dtype=np.float32)
    y = np.empty((B, O2, HW_, HW_), np.float32)
    y[:] = conv_b.reshape(1, O2, 1, 1)
    base = np.arange(PI) * STRIDE
    for c in range(NCORES):
        z = z_list[c]  # (128, SLOTS, 576)
        for t in range(VALID[c]):
            s = S_LISTS[c][t]
            pr, pc = divmod(s, PP)
            blk = z[:, t, :].reshape(O2, B, PI, PI).transpose(1, 0, 2, 3)
            y[:, :, (base + pr)[:, None], (base + pc)[None, :]] = blk
    return y.astype(out_dtype, copy=False)


def _run(inputs, trace=False, trace_kwargs=None, dtype=DTYPE):
    from concourse.bass_utils import run_bass_kernel_spmd

    in_maps = _prep_in_maps(
        inputs["x_kv"], inputs["Wv"], inputs["bv"],
        inputs["conv_w"], inputs["conv_b"], dtype=dtype,
    )
    nc = _get_program(dtype)
    kw = {}
    if trace:
        kw["trace"] = True
        if trace_kwargs:
            kw.update(trace_kwargs)
    res = run_bass_kernel_spmd(nc, in_maps, list(range(NCORES)), **kw)
    z_list = [res.results[c]["zout"] for c in range(NCORES)]
    out = _assemble(z_list, inputs["conv_b"])
    return out, res


def kernel(**inputs):
    out, _ = _run(inputs, trace=False)
    return out


# revision 29
# speedup vs baseline: 1.3246x; 1.1875x over previous
# Trainium2 Bass kernel for nn_LocalCrossAttentionModule.
#
# Math: softmax over a size-1 axis is identically 1, so q/k (and x_query,
# Wq, bq, Wk, bk) never affect the output. The module reduces to, per
# 5x5 patch p (576 of them = 4 batch x 12x12 grid, stride 36):
#   kvf_p  = flatten(x_kv patch)                  (3200,)
#   v_p    = Wv @ kvf_p + bv                      (1600,) viewed as (64, 5, 5)
#   z_p    = conv_w @ v_p[:, s] + conv_b          (128,) per pixel s in 5x5
# z_p is scattered into an otherwise-constant (conv_b) output map.
#
# Sharding: the 25 patch pixels s are split across 8 cores (4 slots each,
# 7 junk/dup slots). Every core sees all 576 patches as the matmul moving
# dim (2 chunks of 288 >= 256 keeps float32r matmuls at full rate).
# Host does layout only: patch gather, weight permutation/transpose,
# final scatter into the conv_b-filled canvas.

import numpy as np

B = 4
CKV = 128
HW_ = 432
E = 2
PP = 5          # patch side
STRIDE = 36
PI = 12         # patch grid side
NP = B * PI * PI   # 576 patches
KF = CKV * PP * PP  # 3200 kv features per patch
KT = KF // 128      # 25 contraction tiles
OUT = 64
O2 = 128
SLOTS = 4
M = SLOTS * OUT    # 256 v-features per core
NCHUNK = 288       # patch chunk (2 x 288 = 576)
NCORES = 8

DTYPE = "f16"      # "f32r" (most accurate) | "f16" (half DMA bytes, ~5e-4) | "bf16"

# pixel-slot assignment: cores 0-6 own 3 pixels (4th slot duplicates the
# first), core 7 owns 4.
S_LISTS = [[3 * c, 3 * c + 1, 3 * c + 2, 3 * c] for c in range(7)]
S_LISTS.append([21, 22, 23, 24])
VALID = [3] * 7 + [4]

_PROGRAM = {}


def _build_program(dtype=DTYPE):
    import concourse.mybir as mybir
    from concourse import bacc
    from concourse.tile import TileContext

    f32 = mybir.dt.float32
    half = {"bf16": mybir.dt.bfloat16, "f16": mybir.dt.float16}
    mm_dt = mybir.dt.float32r if dtype == "f32r" else half[dtype]
    # matmul-2 operand dtype: DVE cannot produce float32r, so f32r mode
    # runs the (tiny) second matmul in plain fp32
    v_dt = f32 if dtype == "f32r" else half[dtype]

    WKC = M + NP  # 832 cols per k-tile: [w(256) | kvf(576)]

    nc = bacc.Bacc()
    wk_d = nc.declare_dram_parameter("wk", [128, KT, WKC], mm_dt, isOutput=False)
    cwbc_d = nc.declare_dram_parameter("cwbc", [128, 131], f32, isOutput=False)
    z_d = nc.declare_dram_parameter("zout", [128, SLOTS, NP], f32, isOutput=True)

    with TileContext(nc) as tc:
        with (
            tc.tile_pool(name="consts", bufs=1) as cpool,
            tc.tile_pool(name="wbig", bufs=1) as wpool,
            tc.tile_pool(name="vbuf", bufs=1) as vpool,
            tc.tile_pool(name="zbuf", bufs=1) as zpool,
            tc.tile_pool(name="ps1", bufs=1, space="PSUM") as ps1,
            tc.tile_pool(name="ps2", bufs=3, space="PSUM") as ps2,
            tc.tile_pool(name="ps0", bufs=1, space="PSUM") as ps0,
        ):
            # PE warm-up: dummy matmuls on a zeroed scratch tile keep the
            # PE_HAM activity window busy from t~0 so real matmuls run at
            # 2.4 GHz instead of the cold 1.2 GHz
            warm_t = cpool.tile([128, 512], f32, name="warm_t")
            nc.gpsimd.memset(warm_t[:], 0.0)
            wps = ps0.tile([128, 512], f32, name="wps")
            for _ in range(6):
                nc.tensor.matmul(
                    wps[:], lhsT=warm_t[:, 0:128], rhs=warm_t[:],
                    start=True, stop=True,
                )

            cwbc_t = cpool.tile([128, 131], f32, name="cwbc_t")
            nc.sync.dma_start(cwbc_t[:], cwbc_d[:])
            # DVE-produced copy of conv_w.T so matmul-2 waits only on DVE
            cw_t = cpool.tile([128, 128], v_dt, name="cw_t")
            nc.vector.tensor_copy(cw_t[:], cwbc_t[:, 0:128])

            wk_t = wpool.tile([128, KT, WKC], mm_dt, name="wk_t")
            # chunked loads, small first so the first matmul starts early
            sizes = [1, 1, 2, 2, 3, 3, 3, 3, 3, 4]
            lo = 0
            for sz in sizes:
                nc.sync.dma_start(wk_t[:, lo:lo + sz, :], wk_d[:, lo:lo + sz, :])
                lo += sz

            # matmul 1: V[f, n] = sum_j WvT[j, f] * KVF_T[j, n]
            ps_v = [
                [ps1.tile([128, NCHUNK], f32, name=f"psv{m}{n}") for n in range(2)]
                for m in range(2)
            ]
            for k in range(KT):
                for m in range(2):
                    for n in range(2):
                        nc.tensor.matmul(
                            ps_v[m][n][:],
                            lhsT=wk_t[:, k, m * 128:(m + 1) * 128],
                            rhs=wk_t[:, k, M + n * NCHUNK:M + (n + 1) * NCHUNK],
                            start=(k == 0),
                            stop=(k == KT - 1),
                        )
                # keep-warm filler: PE would otherwise idle waiting for the
                # next k-tile DMA, letting PE_HAM throttle the clock to 1.2GHz.
                # Small moving dim: just enough activity to hold the clock.
                if k % 2 == 0:
                    nc.tensor.matmul(
                        wps[:, 0:128], lhsT=warm_t[:, 0:128],
                        rhs=warm_t[:, 0:128],
                        start=True, stop=True,
                    )

            # V to SBUF (+bv), zero-padded to 128 partitions per pixel-slot
            v_t = []
            for s in range(SLOTS):
                vt = vpool.tile([128, NP], v_dt, name=f"vt{s}")
                nc.vector.memset(vt[64:128, :], 0.0)
                v_t.append(vt)
            for m in range(2):
                for n in range(2):
                    for h in range(2):
                        s = 2 * m + h
                        nc.vector.tensor_tensor(
                            out=v_t[s][0:64, n * NCHUNK:(n + 1) * NCHUNK],
                            in0=ps_v[m][n][h * 64:(h + 1) * 64, :],
                            in1=cwbc_t[h * 64:(h + 1) * 64, 128 + m:129 + m]
                            .to_broadcast((64, NCHUNK)),
                            op=mybir.AluOpType.add,
                        )

            # matmul 2: z[o2, n] = sum_o conv_w[o2, o] * V[s*64+o, n]
            z_t = zpool.tile([128, SLOTS, NP], f32, name="z_t")
            for s in range(SLOTS):
                for n in range(2):
                    nsl = slice(n * NCHUNK, (n + 1) * NCHUNK)
                    psz = ps2.tile([128, NCHUNK], f32, name="psz")
                    nc.tensor.matmul(
                        psz[:],
                        lhsT=cw_t[:],
                        rhs=v_t[s][:, nsl],
                        start=True,
                        stop=True,
                    )
                    nc.vector.tensor_tensor(
                        out=z_t[:, s, nsl],
                        in0=psz[:],
                        in1=cwbc_t[:, 130:131].to_broadcast((128, NCHUNK)),
                        op=mybir.AluOpType.add,
                    )
                    # store each chunk as soon as it is ready
                    nc.sync.dma_start(z_d[:, s, nsl], z_t[:, s, nsl])
    nc.finalize()
    return nc


def _get_program(dtype=DTYPE):
    if dtype not in _PROGRAM:
        _PROGRAM[dtype] = _build_program(dtype)
    return _PROGRAM[dtype]


def _round_fp32r(a):
    """Round fp32 array to the FP32R grid (12-bit mantissa): (u+0x800)&~0xfff."""
    u = np.ascontiguousarray(a, dtype=np.float32).view(np.uint32)
    u = (u + np.uint32(0x800)) & np.uint32(0xFFFFF000)
    return u.view(np.float32)


def _mm_cast(a, dtype):
    if dtype == "f32r":
        return _round_fp32r(a)
    if dtype == "f16":
        return np.ascontiguousarray(a, dtype=np.float32).astype(np.float16)
    import ml_dtypes

    return np.ascontiguousarray(a, dtype=np.float32).astype(ml_dtypes.bfloat16)


def _prep_in_maps(x_kv, Wv, bv, conv_w, conv_b, dtype=DTYPE):
    """Host-side shard/layout prep. Returns list of per-core input dicts."""
    x_kv = np.ascontiguousarray(np.asarray(x_kv, dtype=np.float32))
    Wv = np.asarray(Wv, dtype=np.float32)
    bv = np.asarray(bv, dtype=np.float32)
    conv_w = np.asarray(conv_w, dtype=np.float32)
    conv_b = np.asarray(conv_b, dtype=np.float32)

    # gather all 5x5 patches (padded coords: top-left of patch (pi,pj) is
    # original coords (pi*36-2, pj*36-2))
    pad = np.zeros((B, CKV, HW_ + 2 * E, HW_ + 2 * E), np.float32)
    pad[:, :, E:HW_ + E, E:HW_ + E] = x_kv
    r = (np.arange(PI)[:, None] * STRIDE + np.arange(PP)).ravel()  # (60,)
    g = pad[:, :, r[:, None], r[None, :]]                # (B, C, 60, 60)
    g = g.reshape(B, CKV, PI, PP, PI, PP)
    # feature j = c*25 + pr*5 + pc ; patch n = b*144 + pi*12 + pj
    kvf_t = g.transpose(1, 3, 5, 0, 2, 4).reshape(KF, NP)   # (3200, 576)
    # device layout [partition, k-tile, patch]
    kvf_arr = kvf_t.reshape(KT, 128, NP).transpose(1, 0, 2)

    cw = np.zeros((128, 128), np.float32)
    cw[:OUT, :] = conv_w.T  # cw[o, o2] = conv_w[o2, o]

    in_maps = []
    for c in range(NCORES):
        perm = np.array(
            [o * PP * PP + s for s in S_LISTS[c] for o in range(OUT)], np.int64
        )
        wv_t = Wv[perm].T                      # (3200, 256)
        wv_arr = wv_t.reshape(KT, 128, M).transpose(1, 0, 2)
        # single blob: per k-tile [w(256) | kvf(576)]
        wk = np.concatenate([wv_arr, kvf_arr], axis=2)  # (128, 25, 832)
        wk = _mm_cast(wk, dtype)
        # f32 consts blob: [cw(128) | bv(2) | cb(1)]
        cwbc = np.empty((128, 131), np.float32)
        cwbc[:, 0:128] = cw
        cwbc[:, 128:130] = bv[perm].reshape(2, 128).T
        cwbc[:, 130] = conv_b
        in_maps.append({"wk": wk, "cwbc": cwbc})
    return in_maps


def _assemble(z_list, conv_b, out_dtype=np.float32):
    """Scatter per-core z outputs into the full (B, 128, 432, 432) map."""
    conv_b = np.asarray(conv_b, dtype=np.float32)
    y = np.empty((B, O2, HW_, HW_), np.float32)
    y[:] = conv_b.reshape(1, O2, 1, 1)
    base = np.arange(PI) * STRIDE
    for c in range(NCORES):
        z = z_list[c]  # (128, SLOTS, 576)
        for t in range(VALID[c]):
            s = S_LISTS[c][t]
            pr, pc = divmod(s, PP)
            blk = z[:, t, :].reshape(O2, B, PI, PI).transpose(1, 0, 2, 3)
            y[:, :, (base + pr)[:, None], (base + pc)[None, :]] = blk
    return y.astype(out_dtype, copy=False)


def _run(inputs, trace=False, trace_kwargs=None, dtype=DTYPE):
    from concourse.bass_utils import run_bass_kernel_spmd

    in_maps = _prep_in_maps(
        inputs["x_kv"], inputs["Wv"], inputs["bv"],
        inputs["conv_w"], inputs["conv_b"], dtype=dtype,
    )
    nc = _get_program(dtype)
    kw = {}
    if trace:
        kw["trace"] = True
        if trace_kwargs:
            kw.update(trace_kwargs)
    res = run_bass_kernel_spmd(nc, in_maps, list(range(NCORES)), **kw)
    z_list = [res.results[c]["zout"] for c in range(NCORES)]
    out = _assemble(z_list, inputs["conv_b"])
    return out, res


def kernel(**inputs):
    out, _ = _run(inputs, trace=False)
    return out


# revision 30
# speedup vs baseline: 1.3392x; 1.0110x over previous
# Trainium2 Bass kernel for nn_LocalCrossAttentionModule.
#
# Math: softmax over a size-1 axis is identically 1, so q/k (and x_query,
# Wq, bq, Wk, bk) never affect the output. The module reduces to, per
# 5x5 patch p (576 of them = 4 batch x 12x12 grid, stride 36):
#   kvf_p  = flatten(x_kv patch)                  (3200,)
#   v_p    = Wv @ kvf_p + bv                      (1600,) viewed as (64, 5, 5)
#   z_p    = conv_w @ v_p[:, s] + conv_b          (128,) per pixel s in 5x5
# z_p is scattered into an otherwise-constant (conv_b) output map.
#
# Sharding: the 25 patch pixels s are split across 8 cores (4 slots each,
# 7 junk/dup slots). Every core sees all 576 patches as the matmul moving
# dim (2 chunks of 288 >= 256 keeps float32r matmuls at full rate).
# Host does layout only: patch gather, weight permutation/transpose,
# final scatter into the conv_b-filled canvas.

import numpy as np

B = 4
CKV = 128
HW_ = 432
E = 2
PP = 5          # patch side
STRIDE = 36
PI = 12         # patch grid side
NP = B * PI * PI   # 576 patches
KF = CKV * PP * PP  # 3200 kv features per patch
KT = KF // 128      # 25 contraction tiles
OUT = 64
O2 = 128
SLOTS = 4
M = SLOTS * OUT    # 256 v-features per core
NCHUNK = 288       # patch chunk (2 x 288 = 576)
NCORES = 8

DTYPE = "f16"      # "f32r" (most accurate) | "f16" (half DMA bytes, ~5e-4) | "bf16"

# pixel-slot assignment: cores 0-6 own 3 pixels (4th slot duplicates the
# first), core 7 owns 4.
S_LISTS = [[3 * c, 3 * c + 1, 3 * c + 2, 3 * c] for c in range(7)]
S_LISTS.append([21, 22, 23, 24])
VALID = [3] * 7 + [4]

_PROGRAM = {}


def _build_program(dtype=DTYPE):
    import concourse.mybir as mybir
    from concourse import bacc
    from concourse.tile import TileContext

    f32 = mybir.dt.float32
    half = {"bf16": mybir.dt.bfloat16, "f16": mybir.dt.float16}
    mm_dt = mybir.dt.float32r if dtype == "f32r" else half[dtype]
    # matmul-2 operand dtype: DVE cannot produce float32r, so f32r mode
    # runs the (tiny) second matmul in plain fp32
    v_dt = f32 if dtype == "f32r" else half[dtype]

    WKC = M + NP  # 832 cols per k-tile: [w(256) | kvf(576)]

    nc = bacc.Bacc()
    wk_d = nc.declare_dram_parameter("wk", [128, KT, WKC], mm_dt, isOutput=False)
    cwbc_d = nc.declare_dram_parameter("cwbc", [128, 131], f32, isOutput=False)
    z_d = nc.declare_dram_parameter("zout", [128, SLOTS, NP], f32, isOutput=True)

    with TileContext(nc) as tc:
        with (
            tc.tile_pool(name="consts", bufs=1) as cpool,
            tc.tile_pool(name="wbig", bufs=1) as wpool,
            tc.tile_pool(name="vbuf", bufs=1) as vpool,
            tc.tile_pool(name="zbuf", bufs=1) as zpool,
            tc.tile_pool(name="ps1", bufs=1, space="PSUM") as ps1,
            tc.tile_pool(name="ps2", bufs=3, space="PSUM") as ps2,
            tc.tile_pool(name="ps0", bufs=1, space="PSUM") as ps0,
        ):
            # PE warm-up: dummy matmuls on a zeroed scratch tile keep the
            # PE_HAM activity window busy from t~0 so real matmuls run at
            # 2.4 GHz instead of the cold 1.2 GHz
            warm_t = cpool.tile([128, 512], f32, name="warm_t")
            nc.gpsimd.memset(warm_t[:], 0.0)
            wps = ps0.tile([128, 512], f32, name="wps")
            for _ in range(4):
                nc.tensor.matmul(
                    wps[:], lhsT=warm_t[:, 0:128], rhs=warm_t[:],
                    start=True, stop=True,
                )

            cwbc_t = cpool.tile([128, 131], f32, name="cwbc_t")
            nc.sync.dma_start(cwbc_t[:], cwbc_d[:])
            # DVE-produced copy of conv_w.T so matmul-2 waits only on DVE
            cw_t = cpool.tile([128, 128], v_dt, name="cw_t")
            nc.vector.tensor_copy(cw_t[:], cwbc_t[:, 0:128])

            wk_t = wpool.tile([128, KT, WKC], mm_dt, name="wk_t")
            # chunked loads, small first so the first matmul starts early
            sizes = [1, 2, 3, 3, 4, 4, 4, 4]
            lo = 0
            for sz in sizes:
                nc.sync.dma_start(wk_t[:, lo:lo + sz, :], wk_d[:, lo:lo + sz, :])
                lo += sz

            # matmul 1: V[f, n] = sum_j WvT[j, f] * KVF_T[j, n]
            ps_v = [
                [ps1.tile([128, NCHUNK], f32, name=f"psv{m}{n}") for n in range(2)]
                for m in range(2)
            ]
            for k in range(KT):
                for m in range(2):
                    for n in range(2):
                        nc.tensor.matmul(
                            ps_v[m][n][:],
                            lhsT=wk_t[:, k, m * 128:(m + 1) * 128],
                            rhs=wk_t[:, k, M + n * NCHUNK:M + (n + 1) * NCHUNK],
                            start=(k == 0),
                            stop=(k == KT - 1),
                        )
                # keep-warm filler: PE would otherwise idle waiting for the
                # next k-tile DMA, letting PE_HAM throttle the clock to 1.2GHz.
                # Small moving dim: just enough activity to hold the clock.
                if k % 2 == 0:
                    nc.tensor.matmul(
                        wps[:, 0:128], lhsT=warm_t[:, 0:128],
                        rhs=warm_t[:, 0:128],
                        start=True, stop=True,
                    )

            # V to SBUF (+bv), zero-padded to 128 partitions per pixel-slot
            v_t = []
            for s in range(SLOTS):
                vt = vpool.tile([128, NP], v_dt, name=f"vt{s}")
                nc.vector.memset(vt[64:128, :], 0.0)
                v_t.append(vt)
            for m in range(2):
                for n in range(2):
                    for h in range(2):
                        s = 2 * m + h
                        nc.vector.tensor_tensor(
                            out=v_t[s][0:64, n * NCHUNK:(n + 1) * NCHUNK],
                            in0=ps_v[m][n][h * 64:(h + 1) * 64, :],
                            in1=cwbc_t[h * 64:(h + 1) * 64, 128 + m:129 + m]
                            .to_broadcast((64, NCHUNK)),
                            op=mybir.AluOpType.add,
                        )

            # matmul 2: z[o2, n] = sum_o conv_w[o2, o] * V[s*64+o, n]
            z_t = zpool.tile([128, SLOTS, NP], f32, name="z_t")
            for s in range(SLOTS):
                for n in range(2):
                    nsl = slice(n * NCHUNK, (n + 1) * NCHUNK)
                    psz = ps2.tile([128, NCHUNK], f32, name="psz")
                    nc.tensor.matmul(
                        psz[:],
                        lhsT=cw_t[:],
                        rhs=v_t[s][:, nsl],
                        start=True,
                        stop=True,
                    )
                    nc.vector.tensor_tensor(
                        out=z_t[:, s, nsl],
                        in0=psz[:],
                        in1=cwbc_t[:, 130:131].to_broadcast((128, NCHUNK)),
                        op=mybir.AluOpType.add,
                    )
                    # store each chunk as soon as it is ready
                    nc.sync.dma_start(z_d[:, s, nsl], z_t[:, s, nsl])
    nc.finalize()
    return nc


def _get_program(dtype=DTYPE):
    if dtype not in _PROGRAM:
        _PROGRAM[dtype] = _build_program(dtype)
    return _PROGRAM[dtype]


def _round_fp32r(a):
    """Round fp32 array to the FP32R grid (12-bit mantissa): (u+0x800)&~0xfff."""
    u = np.ascontiguousarray(a, dtype=np.float32).view(np.uint32)
    u = (u + np.uint32(0x800)) & np.uint32(0xFFFFF000)
    return u.view(np.float32)


def _mm_cast(a, dtype):
    if dtype == "f32r":
        return _round_fp32r(a)
    if dtype == "f16":
        return np.ascontiguousarray(a, dtype=np.float32).astype(np.float16)
    import ml_dtypes

    return np.ascontiguousarray(a, dtype=np.float32).astype(ml_dtypes.bfloat16)


def _prep_in_maps(x_kv, Wv, bv, conv_w, conv_b, dtype=DTYPE):
    """Host-side shard/layout prep. Returns list of per-core input dicts."""
    x_kv = np.ascontiguousarray(np.asarray(x_kv, dtype=np.float32))
    Wv = np.asarray(Wv, dtype=np.float32)
    bv = np.asarray(bv, dtype=np.float32)
    conv_w = np.asarray(conv_w, dtype=np.float32)
    conv_b = np.asarray(conv_b, dtype=np.float32)

    # gather all 5x5 patches (padded coords: top-left of patch (pi,pj) is
    # original coords (pi*36-2, pj*36-2))
    pad = np.zeros((B, CKV, HW_ + 2 * E, HW_ + 2 * E), np.float32)
    pad[:, :, E:HW_ + E, E:HW_ + E] = x_kv
    r = (np.arange(PI)[:, None] * STRIDE + np.arange(PP)).ravel()  # (60,)
    g = pad[:, :, r[:, None], r[None, :]]                # (B, C, 60, 60)
    g = g.reshape(B, CKV, PI, PP, PI, PP)
    # feature j = c*25 + pr*5 + pc ; patch n = b*144 + pi*12 + pj
    kvf_t = g.transpose(1, 3, 5, 0, 2, 4).reshape(KF, NP)   # (3200, 576)
    # device layout [partition, k-tile, patch]
    kvf_arr = kvf_t.reshape(KT, 128, NP).transpose(1, 0, 2)

    cw = np.zeros((128, 128), np.float32)
    cw[:OUT, :] = conv_w.T  # cw[o, o2] = conv_w[o2, o]

    in_maps = []
    for c in range(NCORES):
        perm = np.array(
            [o * PP * PP + s for s in S_LISTS[c] for o in range(OUT)], np.int64
        )
        wv_t = Wv[perm].T                      # (3200, 256)
        wv_arr = wv_t.reshape(KT, 128, M).transpose(1, 0, 2)
        # single blob: per k-tile [w(256) | kvf(576)]
        wk = np.concatenate([wv_arr, kvf_arr], axis=2)  # (128, 25, 832)
        wk = _mm_cast(wk, dtype)
        # f32 consts blob: [cw(128) | bv(2) | cb(1)]
        cwbc = np.empty((128, 131), np.float32)
        cwbc[:, 0:128] = cw
        cwbc[:, 128:130] = bv[perm].reshape(2, 128).T
        cwbc[:, 130] = conv_b
        in_maps.append({"wk": wk, "cwbc": cwbc})
    return in_maps


def _assemble(z_list, conv_b, out_dtype=np.float32):
    """Scatter per-core z outputs into the full (B, 128, 432, 432) map."""
    conv_b = np.asarray(conv_b, dtype=np.float32)
    y = np.empty((B, O2, HW_, HW_), np.float32)
    y[:] = conv_b.reshape(1, O2, 1, 1)
    base = np.arange(PI) * STRIDE
    for c in range(NCORES):
        z = z_list[c]  # (128, SLOTS, 576)
        for t in range(VALID[c]):
            s = S_LISTS[c][t]
            pr, pc = divmod(s, PP)
            blk = z[:, t, :].reshape(O2, B, PI, PI).transpose(1, 0, 2, 3)
            y[:, :, (base + pr)[:, None], (base + pc)[None, :]] = blk
    return y.astype(out_dtype, copy=False)


def _run(inputs, trace=False, trace_kwargs=None, dtype=DTYPE):
    from concourse.bass_utils import run_bass_kernel_spmd

    in_maps = _prep_in_maps(
        inputs["x_kv"], inputs["Wv"], inputs["bv"],
        inputs["conv_w"], inputs["conv_b"], dtype=dtype,
    )
    nc = _get_program(dtype)
    kw = {}
    if trace:
        kw["trace"] = True
        if trace_kwargs:
            kw.update(trace_kwargs)
    res = run_bass_kernel_spmd(nc, in_maps, list(range(NCORES)), **kw)
    z_list = [res.results[c]["zout"] for c in range(NCORES)]
    out = _assemble(z_list, inputs["conv_b"])
    return out, res


def kernel(**inputs):
    out, _ = _run(inputs, trace=False)
    return out
